# revision 60
# baseline (speedup 1.0000x reference)
"""Multi-head causal self-attention (B=1, S=4096, D=1024, H=16) on 8 TRN2 cores.

Sharding: 2 heads per core (head/tensor parallel). Each core computes its
heads' Q/K/V projections, causal flash attention, and a partial output
projection against its 128 columns of Wo. The host sums the 8 partials and
adds the output bias.

Device layouts (per core, bf16 compute):
  - x is fed transposed:  xT [D=1024, S=4096]   (model dim on partitions)
  - Q^T, K^T [128, 4096]: per-core head dims on partitions (h0: 0-63, h1: 64-127)
  - V natural [4096, 130]: per seq-tile [128, 65*2] = [V_h0 | ones | V_h1 | ones]
    The ones column makes the PV matmul also produce the softmax denominator.
  - scores are computed transposed S^T[k, q] so the PV matmul needs no
    transposition; softmax is exp-only (scores are bounded, no max-subtract).
  - output is written transposed outT [1024, 4096] bf16 (partial; host sums
    the 8 cores' partials in fp32 and adds the output bias).

Schedule notes (v2, ~216us vs the ~231us v1):
  - The kernel is PE-bound (~193us PE-busy vs ~135us ACT): the schedule's job
    is to keep the PE matmul stream gapless and the serial head/tail short.
  - The PE ping-pongs between the 64x128 row-tiled mode (score pairs: the
    two heads' K=64 matmuls run concurrently — bass auto-derives
    tile_position from the operands' base partitions) and the plain 128x128
    mode (PV, projections, V transposes). Each mode transition costs a
    ~100-250ns LDWEIGHTS/drain bubble, so k-tiles are processed in PAIRS:
    one 64-mode stretch (2 score pairs) then one 128-mode stretch (pacing
    items first — their LDWEIGHTS absorb the switch bubble — then 2 PV
    pairs) -> half the mode switches of a per-tile interleave.
  - PVs lag TWO pairs behind their scores so the PE never stalls on the exp
    conveyor; projection/oproj pacing items are spread via global per-block
    queues with deadlines (kv(b) before block b's diagonal, q(b) before its
    first score), and oproj (deadline-free) is pushed out of the PE-tight
    early blocks into the later blocks' slack.
  - DMA: the sync HW queue runs ~250-400GB/s, the scalar HW queue only
    ~80GB/s, the gpsimd queue is slow SWDGE. The 16 HW DMA engines split
    bandwidth over everything enqueued, so bulk x is issued just-in-time
    (x(b+2) from inside block b) and the head enqueues only x0/x1 + weights
    (biases ride inside wq's transfer to avoid 128 tiny descriptors).
    Output tiles are written in m-tile pairs (one 2KB-descriptor DMA each).
  - A fraction of late-block full-tile exp slabs runs on the Vector engine
    via the bf16 fast-exp bit trick (int16 = round(A*s + B) bitcast to
    bf16, ~2% rms on those weights; the ones-column denominator shares the
    approximation so most of the error cancels in the softmax ratio).
  - Warmup matmuls on a memset dummy run at kernel start (no DMA dep) so
    the PE HAM clock-gate is released before the first projection; a dummy
    exp preloads the ACT table set; the tail block's normalize uses idle
    score slabs + the idle ACT for its reciprocal broadcast chain.
"""

import numpy as np
import ml_dtypes
from contextlib import ExitStack

import concourse.bass as bass
import concourse.tile as tile
from concourse import bacc, mybir
from concourse.bass_utils import run_bass_kernel_spmd

P = 128
S = 4096
D = 1024
DH = 64
N_CORES = 8
SCALE = 1.0 / 8.0  # 1/sqrt(64)
NQ = 512           # query block (matmul free dim)
KT = 128           # key tile (contraction partitions)
NQB = S // NQ      # 8 query blocks
NKT = S // KT      # 32 key tiles
KO = D // P        # 8 contraction subtiles over the model dim

# natural block order: the head-critical x bytes are just x0 (1MB); the tail
# cost is mostly block-size-independent (normalize + 8 oproj + drain)
ORDER = [0, 1, 2, 3, 4, 5, 6, 7]

# fast-exp offload: in late (ACT-bound) blocks >= FE_MIN_B, full tiles with
# kt % FE_STRIDE == FE_PHASE run their exp on the Vector engine (bit trick,
# ~2% rms on those weights; ~13/144 slabs -> ~0.6% output contribution).
# FE_STRIDE = 0 disables.
FE_STRIDE = 3
FE_PHASE = 2
FE_MIN_B = 6
# bf16 bits of exp(SCALE*s) ~= round(A*s + B):  A = 128*log2(e)*SCALE
FE_A = 128.0 * 1.4426950408889634 * SCALE
FE_C = 0.0440            # mantissa correction (min max-rel-err on |s/8|<=3)
FE_B = 128.0 * (127.0 - FE_C)

BF16 = mybir.dt.bfloat16
F32 = mybir.dt.float32
I16 = mybir.dt.int16
EXP = mybir.ActivationFunctionType.Exp
ADD = mybir.AluOpType.add
MULT = mybir.AluOpType.mult


def _emit(tc, xT, wqT, wkT, wvT, woT, masks, outT, dbg=None):
    nc = tc.nc
    with ExitStack() as ctx:
        from collections import deque
        from concourse.masks import make_identity

        const = ctx.enter_context(tc.tile_pool(name="const", bufs=1))

        # HAM warmup on a memset dummy: no DMA dependency, so the PE clock
        # un-throttles during the input DMA window.
        warm_sb = const.tile([P, P], BF16)
        nc.vector.memset(warm_sb, 0.25)
        with tc.tile_pool(name="warm_psum", bufs=1, space="PSUM") as wpool:
            wt = wpool.tile([P, P], F32)
            for _ in range(52):
                nc.tensor.matmul(wt, lhsT=warm_sb, rhs=warm_sb, start=True, stop=True)

        # DMA head plan (measured: sync HW queue ~250-400GB/s, scalar HW queue
        # only ~80GB/s, gpsimd SWDGE bursts early then crawls; a transfer's
        # 128-descriptor dispatch costs ~2-6us per queue, so the biases ride
        # inside wq's DMA instead of paying their own 128 tiny descriptors):
        #   sync:   x0 (2 need-order halves), x1, then JIT x(b+2) in-loop
        #   scalar: wq(+biases), wk, wv
        #   gpsimd: masks, wo
        xT_sb = const.tile([P, NQB, KO, NQ], BF16)
        wq_sb = const.tile([P, KO * P + 4], BF16)
        nc.scalar.dma_start(wq_sb, wqT)
        wk_sb = const.tile([P, KO, P], BF16)
        wv_sb = const.tile([P, KO, P], BF16)
        nc.sync.dma_start(xT_sb[:, 0, 0:4], xT[:, 0, 0:4])
        nc.sync.dma_start(wk_sb, wkT)
        nc.sync.dma_start(xT_sb[:, 0, 4:8], xT[:, 0, 4:8])
        nc.sync.dma_start(wv_sb, wvT)
        nc.sync.dma_start(xT_sb[:, 1], xT[:, 1])
        masks_sb = const.tile([P, 4, NQ], BF16)
        nc.gpsimd.dma_start(masks_sb, masks)
        wo_sb = const.tile([P, D], BF16)
        nc.gpsimd.dma_start(wo_sb, woT)
        wqv = wq_sb[:, 0:KO * P].rearrange("p (ko m) -> p ko m", m=P)
        bqk_bf = wq_sb[:, KO * P:KO * P + 3]
        # preload the exp table set (~2.7us) while the input DMAs land
        twarm = const.tile([1, 1], F32)
        nc.vector.memset(twarm, 0.0)
        nc.scalar.activation(twarm, twarm, EXP, scale=1.0)

        qT_sb = const.tile([P, S], BF16)
        kT_sb = const.tile([P, S], BF16)
        vT_sb = const.tile([P, S], BF16)
        v_sb = const.tile([P, S // P, 130], BF16)
        attnT_sb = const.tile([P, S], BF16)
        nc.vector.memset(v_sb, 1.0)  # presets the two ones-columns

        ident = const.tile([P, P], BF16)
        make_identity(nc, ident)
        ones_bf = const.tile([1, DH], BF16)
        nc.vector.memset(ones_bf, 1.0)  # K=1 broadcast lhsT for the tail block

        # PSUM budget (8 banks): spool 4 (two [128,2,512] score slabs),
        # vpool 2 (pv0/pv1 accumulators), ppool 1, opool 1 (pacing banks:
        # proj accum / V transpose / the two halves of a row-tiled oproj).
        spool = ctx.enter_context(tc.tile_pool(name="score_psum", bufs=2, space="PSUM"))
        vpool = ctx.enter_context(tc.tile_pool(name="pv_psum", bufs=1, space="PSUM"))
        ppool = ctx.enter_context(tc.tile_pool(name="proj_psum", bufs=1, space="PSUM"))
        opool = ctx.enter_context(tc.tile_pool(name="oproj_psum", bufs=1, space="PSUM"))
        work = ctx.enter_context(tc.tile_pool(name="work", bufs=7))
        nwork = ctx.enter_context(tc.tile_pool(name="nwork", bufs=3))
        dpool = ctx.enter_context(tc.tile_pool(name="dscratch", bufs=2, space="DRAM"))

        def proj_chunk(bcol, w_sb, dst, n, pool_sel=None):
            """Four pacing items of 2 accumulation matmuls each (shared psum).
            Fine granularity keeps the PE FIFO from starving the exp conveyor."""
            state = {}
            pool, tagn = pool_sel or (ppool, "ps")

            def item(j):
                def emit():
                    if j == 0:
                        ps = pool.tile([P, NQ], F32, tag=tagn, name=f"ps_{bcol}_{n}")
                        state["ps"] = ps
                    ps = state["ps"]
                    for kt in range(2 * j, 2 * j + 2):
                        nc.tensor.matmul(
                            ps,
                            lhsT=w_sb[:, kt, :],
                            rhs=xT_sb[:, n, kt, :],
                            start=(kt == 0),
                            stop=(kt == KO - 1),
                        )
                    if j == KO // 2 - 1:
                        nc.vector.tensor_tensor(
                            dst[:, n * NQ:(n + 1) * NQ],
                            ps,
                            bqk_bf[:, bcol:bcol + 1].to_broadcast([P, NQ]),
                            op=ADD,
                        )
                return emit

            return [item(j) for j in range(KO // 2)]

        def v_transpose(t, pool_sel=None):
            pool, tagn = pool_sel or (ppool, "ps")

            def emit():
                tp = pool.tile([P, P], BF16, tag=tagn, name=f"tp_{t}")
                nc.tensor.transpose(tp, vT_sb[:, t * P:(t + 1) * P], ident)
                nc.vector.tensor_copy(
                    v_sb[:, t, :].rearrange("p (h x) -> p h x", x=65)[:, :, 0:DH],
                    tp.rearrange("p (h x) -> p h x", x=DH),
                )
            return emit

        PP, OP = (ppool, "ps"), (opool, "po")

        def q_items(nb):
            # deadline: emitted before block nb's first score
            return proj_chunk(0, wqv, qT_sb, nb, pool_sel=PP)

        def kv_items(nb):
            # deadline: emitted before block nb's diagonal key tiles (kt=4nb)
            ops = []
            ops += proj_chunk(1, wk_sb, kT_sb, nb, pool_sel=OP)
            ops += proj_chunk(2, wv_sb, vT_sb, nb, pool_sel=PP)
            ops += [v_transpose(t, pool_sel=(OP if t % 2 == 0 else PP))
                    for t in range(4 * nb, 4 * nb + 4)]
            return ops

        ot_state = {}

        def oproj_mtile(b, m, alt=False):
            def emit():
                qsl = slice(b * NQ, (b + 1) * NQ)
                if alt:  # tail: rotate over 4 psum banks (proj/pv rings are idle)
                    pool, tagn = [(opool, "po"), (ppool, "ps"),
                                  (vpool, "pv0"), (vpool, "pv1")][m % 4]
                else:
                    pool, tagn = opool, "po"
                po = pool.tile([P, NQ], F32, tag=tagn, name=f"po_{b}_{m}")
                nc.tensor.matmul(
                    po,
                    lhsT=wo_sb[:, m * P:(m + 1) * P],
                    rhs=attnT_sb[:, qsl],
                    start=True,
                    stop=True,
                )
                # two consecutive m-tiles share one ot tile and one outT DMA
                # (p-major DRAM layout -> 128 descriptors of 2KB, not 2x 1KB)
                if m % 2 == 0:
                    ot_state["ot"] = work.tile([P, 2, NQ], BF16, tag="ot",
                                               name=f"ot_{b}_{m}")
                ot = ot_state["ot"]
                if alt and m % 2 == 1:  # tail: ACT is idle, split evacuation
                    nc.scalar.copy(ot[:, m % 2, :], po)
                else:
                    nc.vector.tensor_copy(ot[:, m % 2, :], po)
                if m % 2 == 1:
                    # late blocks ride the HW queues only (the gpsimd SWDGE
                    # queue drains slowly; its backlog stalls the ot ring);
                    # the tail uses sync+scalar since the ACT queue is idle
                    if alt:
                        dma_eng = nc.sync if m % 4 == 1 else nc.scalar
                    else:
                        dma_eng = nc.sync
                    dma_eng.dma_start(outT[:, b, m // 2], ot)
            return emit

        # upfront: the first block's q projection and block 0's k/v
        for op in q_items(ORDER[0]) + kv_items(0):
            op()
        v0_pend = deque()

        def emit_pv_g(st, pvs, nk):
            pT, kt, q0, nq = st
            for h in (0, 1):
                nc.tensor.matmul(
                    pvs[h][:, q0:],
                    lhsT=v_sb[:, kt, h * 65:(h + 1) * 65],
                    rhs=pT[:, h, :nq],
                    start=(kt == 0),
                    stop=(kt == nk - 1),
                )

        def normalize(b, bi, pvs):
            qsl = slice(b * NQ, (b + 1) * NQ)
            if bi < NQB - 1:
                # normalize via DRAM-bounce denominator broadcast; the round-trip
                # latency hides under the next block's flash loop. Both PSUM
                # copies go first: the next block's first PV reuses these slots
                pvSs = []
                for h in (0, 1):
                    pvS = nwork.tile([DH + 1, NQ], F32, tag="pvS")
                    nc.vector.tensor_copy(pvS, pvs[h])  # frees the PSUM slot
                    pvSs.append(pvS)
                for h in (0, 1):
                    pvS = pvSs[h]
                    # recip_approx only works at base partition 0 on HW: copy
                    # the denominator row down first, invert in place there
                    rcp0 = nwork.tile([1, NQ], F32, tag="rcp0")
                    nc.vector.tensor_copy(rcp0, pvS[DH:DH + 1, :])
                    nc.vector.reciprocal_approx_fast(rcp0, rcp0)
                    scr = dpool.tile([NQ], F32, tag="scr")
                    nc.sync.dma_start(scr, rcp0)
                    rb = nwork.tile([DH, NQ], F32, tag="rb")
                    nc.sync.dma_start(rb, scr[None, :].to_broadcast([DH, NQ]))
                    tmp = nwork.tile([DH, NQ], BF16, tag="tmp")
                    nc.vector.tensor_mul(tmp, pvS[0:DH, :], rb)
                    nc.sync.dma_start(attnT_sb[h * DH:(h + 1) * DH, qsl], tmp)
                    if dbg is not None and b == ORDER[0]:
                        nc.sync.dma_start(dbg[f"pv{h}"], pvS)
            else:
                # tail: no next block to hide DMA latency under — broadcast the
                # denominator with a K=1 matmul instead. h1's whole chain runs
                # first (its attnT needs a partition-shift DMA), its rcb copy
                # rides the idle ACT, and the dbc matmuls use the idle score
                # slabs so they don't contend with draining oproj banks.
                for h in (1, 0):
                    pvS = nwork.tile([DH + 1, NQ], F32, tag="pvS")
                    nc.vector.tensor_copy(pvS, pvs[h])
                    rcp0 = nwork.tile([1, NQ], F32, tag="rcp0")
                    nc.vector.tensor_copy(rcp0, pvS[DH:DH + 1, :])
                    nc.vector.reciprocal_approx_fast(rcp0, rcp0)
                    rcb = nwork.tile([1, NQ], BF16, tag="rcb")
                    if h == 1:
                        nc.scalar.copy(rcb, rcp0)
                    else:
                        nc.vector.tensor_copy(rcb, rcp0)
                    dslab = spool.tile([P, 2, NQ], F32, tag="slab")
                    dbc = dslab[:, 0, :]
                    nc.tensor.matmul(
                        dbc[0:DH, :],
                        lhsT=ones_bf,
                        rhs=rcb,
                        start=True,
                        stop=True,
                    )
                    if h == 0:  # partitions already line up: write attnT directly
                        nc.vector.tensor_mul(
                            attnT_sb[0:DH, qsl], pvS[0:DH, :], dbc[0:DH, :]
                        )
                    else:
                        tmp = nwork.tile([DH, NQ], BF16, tag="tmp")
                        nc.vector.tensor_mul(tmp, pvS[0:DH, :], dbc[0:DH, :])
                        nc.sync.dma_start(attnT_sb[DH:2 * DH, qsl], tmp)
            oproj_q.extend(oproj_mtile(b, m, alt=(bi == NQB - 1))
                           for m in range(D // P))

        # pacing: global queues tagged by block. kv(b)/q(b) must finish before
        # block b's diagonal / first score respectively; both consume x(b), so
        # entries are only popped from the preceding block's diagonal onward
        # (x blocks trickle in over the first ~30us). oproj drains in leftover
        # slack. The previous block's last PVs + normalize are deferred into
        # the next block's first 128-mode stretch so its first scores/exps
        # aren't stalled behind the DVE normalize chain (boundary pipeline).
        kv_q = deque((bb, it) for bb in range(1, NQB) for it in kv_items(bb))
        q_q = deque((bb, it) for bb in range(1, NQB) for it in q_items(bb))
        oproj_q = deque()
        carry = [None]
        for bi, b in enumerate(ORDER):
            nk = 4 * (b + 1)  # causal: only key tiles up to the diagonal

            pvs = [
                vpool.tile([DH + 1, NQ], F32, tag=f"pv{h}", name=f"pv{h}_{b}")
                for h in (0, 1)
            ]

            def emit_score(kt, b=b):
                j = kt - 4 * b  # >= 0 on causal-diagonal key tiles
                q0 = max(0, j) * KT
                nq = NQ - q0
                qs0 = b * NQ + q0
                slab = spool.tile([P, 2, NQ], F32, tag="slab")
                for h in (0, 1):
                    nc.tensor.matmul(
                        slab[:, h, :nq],
                        lhsT=kT_sb[h * DH:(h + 1) * DH, kt * KT:(kt + 1) * KT],
                        rhs=qT_sb[h * DH:(h + 1) * DH, qs0:qs0 + nq],
                        start=True,
                        stop=True,
                    )
                return slab, j, q0, nq

            def emit_exp(slab, kt, j, q0, nq, b=b):
                pT = work.tile([P, 2, NQ], BF16, tag="pT")
                use_dve = (FE_STRIDE > 0 and b >= FE_MIN_B and j < 0
                           and kt % FE_STRIDE == FE_PHASE)
                if use_dve:
                    pti = pT.bitcast(I16)
                    nc.vector.tensor_scalar(
                        pti[:, :, :nq], slab[:, :, :nq],
                        FE_A, FE_B, op0=MULT, op1=ADD,
                    )
                else:
                    nc.scalar.activation(pT[:, :, :nq], slab[:, :, :nq], EXP, scale=SCALE)
                if j >= 0:
                    for h in (0, 1):
                        nc.vector.tensor_mul(
                            pT[:, h, :nq],
                            pT[:, h, :nq],
                            masks_sb[:, j, q0:],
                        )
                return (pT, kt, q0, nq)

            def pop_128(pr, b=b, nk=nk):
                # pacing items for the 128x128-mode stretch (proj, transposes,
                # oproj): two pops per pair-slot plus oproj backlog relief.
                # kv/q entries for block b+1 unlock at block b's diagonal.
                # oproj (deadline-free) is deferred out of the PE-bound early
                # blocks into the ACT-bound late blocks' slack.
                unlocked = b + 1 if (b > 0 and pr >= 2 * b) else b
                omax = 12 if b <= 3 else 6
                for _ in range(2):
                    if kv_q and kv_q[0][0] <= unlocked:
                        kv_q.popleft()[1]()
                    elif q_q and q_q[0][0] <= unlocked:
                        q_q.popleft()[1]()
                    elif oproj_q and (b > 3 or len(oproj_q) > omax):
                        oproj_q.popleft()()
                if oproj_q and len(oproj_q) > omax:
                    oproj_q.popleft()()

            pending_pv = []
            for pr in range(nk // 2):
                kt0, kt1 = 2 * pr, 2 * pr + 1
                if pr == 0 and b + 2 < NQB:
                    # just-in-time bulk x: needed by kv(b+2) one block later
                    nc.sync.dma_start(xT_sb[:, b + 2], xT[:, b + 2])
                if kt0 == 4 * b:  # diagonal needs this block's k/v
                    while kv_q and kv_q[0][0] <= b:
                        kv_q.popleft()[1]()
                # --- 64x128-mode stretch: 2 score pairs ---
                s0 = emit_score(kt0)
                s1 = emit_score(kt1)
                # exp lanes (ACT, or DVE for offloaded full tiles)
                e0 = emit_exp(s0[0], kt0, s0[1], s0[2], s0[3])
                e1 = emit_exp(s1[0], kt1, s1[1], s1[2], s1[3])
                # --- 128x128-mode stretch: PVs lag two pairs behind so the
                # PE never waits on the exp conveyor in ACT-bound stretches ---
                if carry[0] is not None:
                    carry[0]()
                    carry[0] = None
                while v0_pend:
                    v0_pend.popleft()()
                if pr < nk // 2 - 1:
                    # pacing pops first: their LDWEIGHTS absorb the
                    # mode-switch bubble; the lag-2 PVs have slack anyway.
                    # No pops on the last pair so the next block's first
                    # scores (and the exp conveyor) follow immediately.
                    pop_128(pr)
                if len(pending_pv) > 2:
                    for st in pending_pv[:2]:
                        emit_pv_g(st, pvs, nk)
                    pending_pv = pending_pv[2:]
                pending_pv = pending_pv + [e0, e1]
                if pr >= nk // 2 - 2:
                    # q(next) spread over the last two pairs (2 items each) so
                    # the next block's first scores follow a small burst only
                    for _ in range(2):
                        if q_q and q_q[0][0] <= b + 1:
                            q_q.popleft()[1]()

            def fin(pvs=pvs, pending=tuple(pending_pv), b=b, bi=bi, nk=nk):
                for st in pending:
                    emit_pv_g(st, pvs, nk)
                normalize(b, bi, pvs)
            carry[0] = fin
        carry[0]()
        while oproj_q:
            oproj_q.popleft()()
        if dbg is not None:
            nc.sync.dma_start(dbg["qT"], qT_sb)
            nc.sync.dma_start(dbg["kT"], kT_sb)
            nc.sync.dma_start(dbg["v"], v_sb)
            nc.sync.dma_start(dbg["attnT"], attnT_sb)


def build(debug_out=False):
    nc = bacc.Bacc(
        "TRN2",
        target_bir_lowering=False,
        debug=False,
        enable_asserts=False,
    )
    xT = nc.dram_tensor("xT", [P, NQB, KO, NQ], BF16, kind="ExternalInput").ap()
    wqT = nc.dram_tensor("wqT", [P, KO * P + 4], BF16, kind="ExternalInput").ap()
    wkT = nc.dram_tensor("wkT", [P, KO, P], BF16, kind="ExternalInput").ap()
    wvT = nc.dram_tensor("wvT", [P, KO, P], BF16, kind="ExternalInput").ap()
    woT = nc.dram_tensor("woT", [P, D], BF16, kind="ExternalInput").ap()
    masks = nc.dram_tensor("masks", [P, 4, NQ], BF16, kind="ExternalInput").ap()
    outT = nc.dram_tensor("outT", [P, NQB, D // (2 * P), 2, NQ], BF16,
                          kind="ExternalOutput").ap()
    dbg = None
    if debug_out:
        dbg = {
            "qT": nc.dram_tensor("dbg_qT", [P, S], BF16, kind="ExternalOutput").ap(),
            "kT": nc.dram_tensor("dbg_kT", [P, S], BF16, kind="ExternalOutput").ap(),
            "v": nc.dram_tensor("dbg_v", [P, S // P, 130], BF16, kind="ExternalOutput").ap(),
            "attnT": nc.dram_tensor("dbg_attnT", [P, S], BF16, kind="ExternalOutput").ap(),
            "pv0": nc.dram_tensor("dbg_pv0", [DH + 1, NQ], F32, kind="ExternalOutput").ap(),
            "pv1": nc.dram_tensor("dbg_pv1", [DH + 1, NQ], F32, kind="ExternalOutput").ap(),
        }

    with tile.TileContext(nc) as tc:
        _emit(tc, xT, wqT, wkT, wvT, woT, masks, outT, dbg=dbg)
    nc.compile()
    return nc


def _make_masks():
    k = np.arange(P)[:, None]
    q = np.arange(NQ)[None, :]
    m = np.zeros((P, 4, NQ), np.float32)
    for j in range(4):
        m[:, j, :] = ((KT * j + k) <= q).astype(np.float32)
    return m.astype(ml_dtypes.bfloat16)


_STATE = {}


def _prep_inputs(x, Wq, bq, Wk, bk, Wv, bv, Wo, bo):
    """Prepack every input per-partition contiguous so each DMA lowers to 128
    large descriptors (kernel-start latency) instead of thousands of small
    strided reads."""
    bf = ml_dtypes.bfloat16
    # x [S, D] -> [p, n, ko, q] with s = n*NQ+q, d = ko*P+p
    xPre = np.ascontiguousarray(
        np.asarray(x, np.float32).reshape(NQB, NQ, KO, P).transpose(3, 0, 2, 1)
    ).astype(bf)
    masks = _make_masks()
    Wq = np.asarray(Wq, np.float32)
    Wk = np.asarray(Wk, np.float32)
    Wv = np.asarray(Wv, np.float32)
    Wo = np.asarray(Wo, np.float32)
    bq = np.asarray(bq, np.float32)
    bk = np.asarray(bk, np.float32)
    bv = np.asarray(bv, np.float32)

    def wpack(W, r):  # W[r] [m, d] -> [p, ko, m] with d = ko*P+p
        return np.ascontiguousarray(
            W[r].reshape(P, KO, P).transpose(2, 1, 0)
        ).astype(bf)

    in_maps = []
    for c in range(N_CORES):
        r = slice(c * P, (c + 1) * P)
        in_maps.append({
            "xT": xPre,
            "wqT": np.concatenate([
                wpack(Wq, r).reshape(P, KO * P),
                np.stack([bq[r], bk[r], bv[r], np.zeros(P, np.float32)],
                         axis=1).astype(bf),
            ], axis=1),
            "wkT": wpack(Wk, r),
            "wvT": wpack(Wv, r),
            "woT": np.ascontiguousarray(Wo[:, r].T).astype(bf),
            "masks": masks,
        })
    return in_maps


def kernel(x, Wq, bq, Wk, bk, Wv, bv, Wo, bo):
    if "nc" not in _STATE:
        _STATE["nc"] = build()
    nc = _STATE["nc"]
    in_maps = _prep_inputs(x, Wq, bq, Wk, bk, Wv, bv, Wo, bo)
    res = run_bass_kernel_spmd(nc, in_maps, core_ids=list(range(N_CORES)))
    total = res.results[0]["outT"].astype(np.float32)
    for c in range(1, N_CORES):
        total = total + res.results[c]["outT"].astype(np.float32)
    # outT layout [p, b, mpair, i, q]: d = (2*mpair+i)*128+p, s = b*512+q
    outDS = total.transpose(2, 3, 0, 1, 4).reshape(D, S)
    out = outDS.T + np.asarray(bo, np.float32)[None, :]
    return np.ascontiguousarray(out, dtype=np.float32).reshape(1, S, D)
